# revision 20
# baseline (speedup 1.0000x reference)
"""Trainium2 Bass kernel for ClusterMemoryAMP cross-entropy loss.

Computes: loss = 0.5*(ce(hard_logits) + ce(mean_logits)) where
logits = normalize(inputs) @ features.T / 0.05, split in halves of 50000.

Sharding: feature bank [100000, 256] row-sharded across 8 cores
(12500 rows each; cores 0-3 own the "mean" half, 4-7 the "hard" half).

Per core: fp8(e4m3) DoubleRow matmuls (whole K=256 in one instruction)
produce logits*16 in PSUM; the per-row 1/(16*TEMP*|x|) factor is folded
into the exp stage. exp+row-sum is split across the two PSUM-reading
engines: ACT does true exp with fused accumulation on column groups 0-3,
DVE does Schraudolph bit-trick exp + reduce on groups 4-6. Groups are
interleaved A,V,A,V,... so both engines run concurrently. Target logits
come from a gpsimd indirect-DMA gather (bf16) dotted on DVE. Host
combines the tiny per-core partials.
"""

import math

import numpy as np
import orjson

import concourse.bass as bass
import concourse.mybir as mybir
import concourse.tile as tile
from concourse.bass_utils import run_bass_kernel_spmd

# Problem constants (hardcoded per harness contract)
B = 1024  # batch
D = 256  # feature dim
NC = 50000  # clusters per half
M = 8  # cores
ROWS = NC * 2 // M  # 12500 feature rows per core
NPAD = 44  # zero-padded columns per core slab
NCOLS = ROWS + NPAD  # 12544 = 6*2048 + 256
TEMP = 0.05
WSCALE = 16.0  # fp8 weight pre-scale

P = 128
JT = B // P  # 8 batch chunks
KS = D // P  # 2 contraction chunks
MMF = 256  # DoubleRow matmul output free width (rhs free = 2*256 = 512 max)
GW = 1024  # psum tile width (2 banks); 4 tiles in flight = full PSUM
# Column units per j, interleaved ACT/DVE consumers, 4-deep psum pipeline.
# ACT costs ~1.24ns/col (exp + fused accum), the DVE lane ~1.6ns/col
# (Schraudolph to int16/bf16 bits + 16-bit-mode reduce), so ACT gets 7168
# of 12544 columns. Padding columns (12500+) land in the last ACT unit ->
# each pad contributes exactly exp(0)=1.0.
# (c0, width, is_act)
UNITS = []
_c = 0
for _k, _w in enumerate([1024] * 11 + [256, 1024]):
    UNITS.append((_c, _w, _k % 2 == 0))
    _c += _w
assert _c == NCOLS
NGRP = len(UNITS)  # 13

# bf16 Schraudolph exp constants: exp(x) ~= bitcast_bf16(int16(x*2^7/ln2 + B16))
# (top 16 bits of the classic f32 trick; bias tuned so the SUM of exps is
# nearly unbiased for a wide logit spread)
A16 = float((1 << 7) / math.log(2.0))  # 184.665
B16 = float((127 << 7) - 486411 / 65536 + 0.55)
_PAD1 = 1.0  # pads sit in an ACT unit: exp(0 * srow) = 1.0 each

F32 = mybir.dt.float32
BF16 = mybir.dt.bfloat16
FP8 = mybir.dt.float8e4
I32 = mybir.dt.int32
I16 = mybir.dt.int16

_NC_CACHE = None


def _split_multiwait_json(raw: bytes) -> bytes:
    """The walrus build in this container only supports one sync-wait per
    instruction; Tile emits multi-wait instructions (e.g. the tail drain).
    Hoist all-but-the-last wait onto single-wait NoOps on the same engine."""
    m = orjson.loads(raw)
    k = 0
    for f in m["functions"]:
        for bb in f["blocks"]:
            out = []
            for ins in bb["instructions"]:
                si = ins.get("sync_info")
                waits = (si or {}).get("on_wait") or []
                if len(waits) > 1:
                    for w in waits[:-1]:
                        k += 1
                        out.append(
                            {
                                "engine": ins["engine"],
                                "ins": [],
                                "name": f"{ins['name']}-sw{k}",
                                "opcode": "NoOp",
                                "outs": [],
                                "sync_info": {"on_wait": [w], "on_update": []},
                            }
                        )
                    si["on_wait"] = [waits[-1]]
                out.append(ins)
            bb["instructions"] = out
    return orjson.dumps(m)


def _install_json_fix(nc):
    orig = nc.to_json_bytes
    nc.to_json_bytes = lambda: _split_multiwait_json(orig())
    return nc


def _build_nc():
    nc = bass.Bass()

    xt_d = nc.dram_tensor("xt", [P, KS, B], FP8, kind="ExternalInput")
    xs_d = nc.dram_tensor("xs", [P, JT, D], BF16, kind="ExternalInput")
    wt_d = nc.dram_tensor("wt", [P, KS, NCOLS], FP8, kind="ExternalInput")
    wg_d = nc.dram_tensor("wg", [ROWS, D], BF16, kind="ExternalInput")
    tidx_d = nc.dram_tensor("tidx", [P, JT], I32, kind="ExternalInput")
    tmask_d = nc.dram_tensor("tmask", [P, JT], F32, kind="ExternalInput")
    osum_d = nc.dram_tensor("osum", [P, JT], F32, kind="ExternalOutput")
    otgt_d = nc.dram_tensor("otgt", [P, JT], F32, kind="ExternalOutput")

    Exp = mybir.ActivationFunctionType.Exp
    Ln = mybir.ActivationFunctionType.Ln
    AX = mybir.AxisListType.X
    MUL = mybir.AluOpType.mult
    ADD = mybir.AluOpType.add

    with tile.TileContext(nc) as tc:
        with (
            tc.tile_pool(name="const", bufs=1) as const,
            tc.tile_pool(name="scratch", bufs=2) as scratch,
            tc.tile_pool(name="sdvp", bufs=2) as sdvp,
            tc.tile_pool(name="psum", bufs=2, space="PSUM") as psum,
        ):
            xt = const.tile([P, KS, B], FP8, tag="xt")
            xs = const.tile([P, JT, D], BF16, tag="xs")
            wt = const.tile([P, KS, NCOLS], FP8, tag="wt")
            tidx = const.tile([P, JT], I32, tag="tidx")
            tmask = const.tile([P, JT], F32, tag="tmask")
            nsum = const.tile([P, JT], F32, tag="nsum")
            lnn = const.tile([P, JT], F32, tag="lnn")
            srow = const.tile([P, JT], F32, tag="srow")  # 1/(16*T*|x|)
            sA = const.tile([P, JT], F32, tag="sA")  # srow*A32
            st = const.tile([P, JT], F32, tag="st")  # 1/(T*|x|)
            acc = const.tile([P, JT, NGRP], F32, tag="acc")
            gall = const.tile([P, JT, D], BF16, tag="gall")
            tlr = const.tile([P, JT], F32, tag="tlr")
            tl = const.tile([P, JT], F32, tag="tl")
            osum = const.tile([P, JT], F32, tag="osum")

            # ---- input DMAs (xs per-j first so norms can start early) ----
            for j in range(JT):
                nc.sync.dma_start(xs[:, j], xs_d[:, j])
            nc.sync.dma_start(xt[:], xt_d[:])
            nc.sync.dma_start(tidx[:], tidx_d[:])
            nc.sync.dma_start(tmask[:], tmask_d[:])
            for c0, w, _ia in UNITS:
                nc.sync.dma_start(wt[:, :, c0 : c0 + w], wt_d[:, :, c0 : c0 + w])

            # ---- target-row gathers (gpsimd queue, overlaps everything) ----
            for j in range(JT):
                nc.gpsimd.indirect_dma_start(
                    out=gall[:, j],
                    out_offset=None,
                    in_=wg_d[:, :],
                    in_offset=bass.IndirectOffsetOnAxis(ap=tidx[:, j : j + 1], axis=0),
                )

            # ---- norms -> per-row exp scale ----
            # |x|^2 on DVE (bf16 2x mode); srow = exp(-0.5*ln(|x|^2) - ln(16*T))
            lb = math.log(1.0 / (WSCALE * TEMP))
            bias_t = const.tile([P, 1], F32, tag="bias")
            nc.vector.memset(bias_t[:], lb)
            for j in range(JT):
                # |x_j|^2: square on gpsimd, cheap 16-bit reduce on DVE
                sq = scratch.tile([P, D], BF16, tag="sq")
                nc.gpsimd.tensor_tensor(sq[:], xs[:, j], xs[:, j], MUL)
                nc.vector.reduce_sum(nsum[:, j : j + 1], sq[:], axis=AX)
                if j == 0:  # fast path: unblock the first ACT/DVE units
                    nc.scalar.activation(lnn[:, :1], nsum[:, :1], Ln)
                    nc.scalar.activation(
                        srow[:, :1], lnn[:, :1], Exp, bias=bias_t[:], scale=-0.5
                    )
                    nc.vector.tensor_scalar(
                        sA[:, :1], srow[:, :1], A16, None, op0=MUL
                    )
            nc.scalar.activation(lnn[:, 1:], nsum[:, 1:], Ln)
            nc.scalar.activation(srow[:, 1:], lnn[:, 1:], Exp, bias=bias_t[:], scale=-0.5)
            nc.vector.tensor_scalar(sA[:, 1:], srow[:, 1:], A16, None, op0=MUL)

            # ---- main loop: DoubleRow matmuls + split exp/row-sum ----
            def emit_dot(j):
                # target logit raw dot: product on gpsimd, reduce on DVE
                prod = scratch.tile([P, D], BF16, tag="prod")
                nc.gpsimd.tensor_tensor(prod[:], gall[:, j], xs[:, j], MUL)
                nc.vector.reduce_sum(tlr[:, j : j + 1], prod[:], axis=AX)

            for j in range(JT):
                for g, (c0, w, is_act) in enumerate(UNITS):
                    pg = psum.tile([P, GW], F32, tag="pg")
                    for t0 in range(0, w, MMF):
                        nc.tensor.matmul(
                            pg[:, t0 : t0 + MMF],
                            lhsT=xt[:, :, j * P : (j + 1) * P],
                            rhs=wt[:, :, c0 + t0 : c0 + t0 + MMF],
                            start=True,
                            stop=True,
                            perf_mode=mybir.MatmulPerfMode.DoubleRow,
                        )
                    if is_act:
                        # true exp with per-row scale and fused accumulation
                        nc.scalar.activation(
                            pg[:, :w],
                            pg[:, :w],
                            Exp,
                            scale=srow[:, j : j + 1],
                            accum_out=acc[:, j, g : g + 1],
                        )
                    else:
                        # bf16 Schraudolph on DVE (int16 bits), then a
                        # 16-bit-mode reduce of the bf16 view for the row sum
                        sdv = sdvp.tile([P, GW], I16, tag="sdv")
                        nc.vector.tensor_scalar(
                            sdv[:, :w],
                            pg[:, :w],
                            sA[:, j : j + 1],
                            B16,
                            op0=MUL,
                            op1=ADD,
                        )
                        if (g // 2) % 2 == 0:
                            nc.vector.reduce_sum(
                                acc[:, j, g : g + 1],
                                sdv[:, :w].bitcast(BF16),
                                axis=AX,
                            )
                        else:
                            so = sdvp.tile([P, GW], BF16, tag="so")
                            nc.vector.tensor_scalar(
                                so[:, :w],
                                sdv[:, :w].bitcast(BF16),
                                1.0,
                                0.0,
                                op0=MUL,
                                op1=ADD,
                                accum_out=acc[:, j, g : g + 1],
                            )
                if j >= 2:
                    emit_dot(j - 2)
            emit_dot(JT - 2)
            emit_dot(JT - 1)

            # ---- combine partials, target logits ----
            nc.vector.reduce_sum(osum[:], acc[:], axis=AX)
            nc.sync.dma_start(osum_d[:], osum[:])
            nc.vector.tensor_scalar(st[:], srow[:], WSCALE, None, op0=MUL)
            nc.vector.tensor_tensor(tl[:], tlr[:], st[:], MUL)
            nc.vector.tensor_tensor(tl[:], tl[:], tmask[:], MUL)
            nc.sync.dma_start(otgt_d[:], tl[:])

    return _install_json_fix(nc)


def _get_nc():
    global _NC_CACHE
    if _NC_CACHE is None:
        _NC_CACHE = _build_nc()
    return _NC_CACHE


def _prep_in_maps(inputs, targets, features):
    import ml_dtypes

    e4 = ml_dtypes.float8_e4m3
    bf = ml_dtypes.bfloat16

    x = np.asarray(inputs, dtype=np.float32)
    t = np.asarray(targets).astype(np.int64)
    feats = np.asarray(features, dtype=np.float32)

    # shared across cores
    xq = x.astype(e4)  # [B, D] fp8
    # xt[p, s, b] = xq[b, s*128+p]
    xt = np.ascontiguousarray(xq.reshape(B, KS, P).transpose(2, 1, 0))
    xb = x.astype(bf)
    # xs[p, j, d] = xb[j*128+p, d]
    xs = np.ascontiguousarray(xb.reshape(JT, P, D).transpose(1, 0, 2))

    in_maps = []
    for c in range(M):
        half = c // (M // 2)  # 0 = mean half, 1 = hard half
        ci = c % (M // 2)
        r0 = half * NC + ci * ROWS
        slab = feats[r0 : r0 + ROWS]  # [12500, 256]
        wq = (slab * WSCALE).astype(e4)  # [12500, 256] fp8
        wt = np.zeros((P, KS, NCOLS), dtype=e4)
        # wt[p, s, c] = wq[c, s*128+p]
        wt[:, :, :ROWS] = wq.reshape(ROWS, KS, P).transpose(2, 1, 0)
        local = t - ci * ROWS  # target row within this core's slab (per half)
        owned = (local >= 0) & (local < ROWS)
        tidx = np.where(owned, local, 0).astype(np.int32)
        tmask = owned.astype(np.float32)
        # b = j*128 + p -> sbuf [p, j]
        tidx2 = np.ascontiguousarray(tidx.reshape(JT, P).T)
        tmask2 = np.ascontiguousarray(tmask.reshape(JT, P).T)
        in_maps.append(
            {
                "xt": xt,
                "xs": xs,
                "wt": np.ascontiguousarray(wt),
                "wg": np.ascontiguousarray(slab.astype(bf)),
                "tidx": tidx2,
                "tmask": tmask2,
            }
        )
    return in_maps


def _combine(results):
    """results: list of 8 dicts with osum/otgt [128, 8] -> scalar loss."""

    def flat(a):  # [p, j] -> [b] with b = j*128+p
        return np.asarray(a).T.reshape(-1)

    # padding columns live in group 6 (DVE/Schraudolph), each contributing
    # _PAD1 per row per core
    pad = NPAD * _PAD1
    ces = []
    for half in range(2):
        cores = range(half * (M // 2), (half + 1) * (M // 2))
        s = np.zeros(B, dtype=np.float64)
        tlog = np.zeros(B, dtype=np.float64)
        for c in cores:
            s += flat(results[c]["osum"]).astype(np.float64) - pad
            tlog += flat(results[c]["otgt"]).astype(np.float64)
        ces.append(np.mean(np.log(s) - tlog))
    # halves: 0 = mean, 1 = hard; loss = 0.5*(ce(hard)+ce(mean))
    return np.float32(0.5 * (ces[0] + ces[1]))


LAST_RESULT = None  # BassKernelResults of the most recent run (for profiling)


def kernel(inputs, targets, features):
    global LAST_RESULT
    nc = _get_nc()
    in_maps = _prep_in_maps(inputs, targets, features)
    res = run_bass_kernel_spmd(nc, in_maps, core_ids=list(range(M)))
    LAST_RESULT = res
    return _combine(res.results)
